# revision 28
# baseline (speedup 1.0000x reference)
"""Trainium2 Bass kernel for nn_MultiHeadAttention (B=4, S=2048, D=1024, H=16).

Sharding: 8 cores = (batch b in 0..3) x (head-half hb in 0..1).
Each core computes, for its batch b and its 8 heads:
  qT = (Q[b] @ W_q[hb].T).T        [512, S]   (features on partitions)
  kT = (K[b] @ W_k[hb].T).T        [512, S]
  v  =  V[b] @ W_v[hb].T           [S, 512]   (+ ones column per head)
  per head pair: scoresT -> exp -> PV (ones-row gives softmax denominator
  in the same PSUM accumulation), normalize, then out-projection partial.
Host sums the two partial outputs per batch.

One software-pipelined schedule instead of serial phases:
  - prefix: K-proj (DMA-gated), 10/16 V-proj units, Q-proj(c0,p0)
  - main: 16 chunks (sq-chunk-major x head-pair). Score matmuls feed the
    ACT engine's exp (the true bottleneck, ~1.1us per [128,1024] tile);
    PV matmuls lag LAG sk-steps behind. The remaining V units, JIT Q-proj
    units and output-projection units interleave as PE filler.
  - tile_wait_until floors keep the BIR scheduler from hoisting the
    output-projection matmuls behind att writes (its static model
    underestimates the normalize-chain latency, stalling the in-order PE).
  - normalization: po drained to SBUF immediately (frees the single PSUM
    po bank pair), reciprocal_approx_fast (5x faster than reciprocal),
    broadcast + muls on the idle GPSIMD engine.
  - all matmul inputs bf16 (halves DMA; full PE rate), fp32 accumulate.
"""

import sys

sys.path.insert(0, "/opt/trn_rl_repo")

from contextlib import ExitStack

import ml_dtypes
import numpy as np

import concourse.bass as bass  # noqa: F401
import concourse.tile as tile
from concourse import bacc, mybir
from concourse.bass_utils import run_bass_kernel_spmd

F32 = mybir.dt.float32
BF = mybir.dt.bfloat16
EXP = mybir.ActivationFunctionType.Exp

D = 1024  # d_model
HD = 512  # head-dim slice per core (8 heads x 64)
DK = 64
NH = 8  # heads per core
P = 128
S = 2048
SC = 512  # proj streaming chunk of S
JC = 512  # sq chunk per head in attention
KO = D // P  # 8 contraction subtiles
N_SK = S // P  # 16 sk tiles
N_C = S // SC  # 4
LAG = 3  # PV lags exp by this many sk steps

# scheduler placement floors (approx sim-time, us)
PREFIX_US = 45.0
_c = PREFIX_US
EST_START = []
for _x in range(17):
    EST_START.append(_c)
    _c += 21.0 if _x == 0 else (20.0 if _x == 1 else 18.0)

# V units deferred into the chunk loop: {(chunk_X, sk): unit index 10..15}
V_LATE = {(0, 1): 10, (0, 4): 11, (0, 7): 12, (0, 10): 13, (0, 13): 14, (1, 1): 15}


def build():
    nc = bacc.Bacc(
        "TRN2",
        target_bir_lowering=False,
        debug=False,
        enable_asserts=False,
        num_devices=1,
    )

    xqt = nc.dram_tensor("xqt", [D, S], BF, kind="ExternalInput").ap()
    xkt = nc.dram_tensor("xkt", [D, S], BF, kind="ExternalInput").ap()
    xvt = nc.dram_tensor("xvt", [D, S], BF, kind="ExternalInput").ap()
    wqt = nc.dram_tensor("wqt", [D, HD], BF, kind="ExternalInput").ap()
    wkt = nc.dram_tensor("wkt", [D, HD], BF, kind="ExternalInput").ap()
    wvt = nc.dram_tensor("wvt", [D, HD], BF, kind="ExternalInput").ap()
    wot = nc.dram_tensor("wot", [HD, D], BF, kind="ExternalInput").ap()
    out = nc.dram_tensor("out", [S, D], F32, kind="ExternalOutput").ap()

    with tile.TileContext(nc) as tc, ExitStack() as ctx:
        pers = ctx.enter_context(tc.tile_pool(name="pers", bufs=1))
        wpool = ctx.enter_context(tc.tile_pool(name="wpool", bufs=3))
        xpool = ctx.enter_context(tc.tile_pool(name="xpool", bufs=2))
        epool = ctx.enter_context(tc.tile_pool(name="epool", bufs=12))
        pupool = ctx.enter_context(tc.tile_pool(name="pupool", bufs=2))
        rpool = ctx.enter_context(tc.tile_pool(name="rpool", bufs=2))
        bpool = ctx.enter_context(tc.tile_pool(name="bpool", bufs=2))
        tpool = ctx.enter_context(tc.tile_pool(name="tpool", bufs=2))
        opool = ctx.enter_context(tc.tile_pool(name="opool", bufs=3))
        ps_score = ctx.enter_context(tc.tile_pool(name="ps_score", bufs=2, space="PSUM"))
        ps_out = ctx.enter_context(tc.tile_pool(name="ps_out", bufs=1, space="PSUM"))
        ps_small = ctx.enter_context(tc.tile_pool(name="ps_small", bufs=2, space="PSUM"))

        # persistent intermediates. qt/at are split per (c-chunk, pair) tile so
        # concurrent JIT writers and readers get clean whole-tile dependencies.
        kt = pers.tile([P, 4, S], BF)  # kT: [p, pair, s], feature = pair*128+p
        va = pers.tile([P, N_SK, NH, DK + 1], BF)  # [s%128, s//128, h, dv|1]
        qtt = [
            [pers.tile([P, JC], BF, name=f"qt_{c}_{pr}") for pr in range(4)]
            for c in range(N_C)
        ]
        att = [
            [pers.tile([P, JC], BF, name=f"at_{c}_{pr}") for pr in range(4)]
            for c in range(N_C)
        ]

        nc.vector.memset(va[:, :, :, DK], 1.0)

        def load_x(src, c, nm, split=1, tag=None):
            # split>1 loads the chunk as `split` tiles along KO so the first
            # matmuls can start before the whole chunk lands.
            kw = KO // split
            xs = []
            for i in range(split):
                x = xpool.tile(
                    [P, kw, SC], BF, tag=tag or f"x{kw}", name=f"{nm}_{i}",
                    bufs=max(2, split),
                )
                nc.sync.dma_start(
                    x,
                    src[
                        i * kw * P : (i + 1) * kw * P, c * SC : (c + 1) * SC
                    ].rearrange("(o p) s -> p o s", p=P),
                )
                xs.append(x)
            return xs

        def qk_unit(ws, xs, dst, pr):
            ps = ps_small.tile([P, SC], F32, tag="ps", name="ps_qk")
            kw = KO // len(xs)
            kww = KO // len(ws)
            for k in range(KO):
                nc.tensor.matmul(
                    ps,
                    lhsT=ws[k // kww][:, k % kww, pr * P : (pr + 1) * P],
                    rhs=xs[k // kw][:, k % kw, :],
                    start=(k == 0),
                    stop=(k == KO - 1),
                )
            nc.vector.tensor_copy(dst, ps)

        def v_unit(xs, u):
            # unit u computes va[:, u] (sk tile u) from xv chunk u//4
            ps = ps_small.tile([P, HD], F32, tag="ps", name="ps_v")
            kw = KO // len(xs)
            st = u % 4
            for k in range(KO):
                nc.tensor.matmul(
                    ps,
                    lhsT=xs[k // kw][:, k % kw, st * P : (st + 1) * P],
                    rhs=wv[0][:, k, :],
                    start=(k == 0),
                    stop=(k == KO - 1),
                )
            nc.vector.tensor_copy(
                va[:, u, :, 0:DK],
                ps.rearrange("p (h d) -> p h d", d=DK),
            )

        def p3_accum(ps, st, half, prs):
            ci, so = st // 4, (st % 4) * P
            for pr in prs:
                nc.tensor.matmul(
                    ps,
                    lhsT=att[ci][pr][:, so : so + P],
                    rhs=wo[:, pr, half * 512 : (half + 1) * 512],
                    start=(pr == 0),
                    stop=(pr == 3),
                )

        def p3_finish(ps, st, half):
            ob = opool.tile([P, 512], F32, tag="ob", name="ob")
            nc.vector.tensor_copy(ob, ps)
            nc.sync.dma_start(out[st * P : (st + 1) * P, half * 512 : (half + 1) * 512], ob)

        def p3_unit(st, half):
            ps = ps_small.tile([P, 512], F32, tag="ps", name="ps_o")
            p3_accum(ps, st, half, range(4))
            p3_finish(ps, st, half)

        # ---------------- prefix ----------------
        # K-proj (DMA-gated), V units 0-9, Q(c0,p0). The first weight/x loads
        # are split so the PE starts sooner. bufs=2 x tags + the concurrent
        # DMA engine give one-chunk-deep prefetch.
        wk = []
        for i in range(2):
            w = wpool.tile([P, KO // 2, HD], BF, tag="wh", name=f"wk{i}", bufs=2)
            nc.sync.dma_start(
                w, wkt[i * 512 : (i + 1) * 512, :].rearrange("(o p) m -> p o m", p=P)
            )
            wk.append(w)
        for c in range(N_C):
            xk = load_x(xkt, c, f"xk{c}", split=4 if c == 0 else 1)
            for pr in range(4):
                qk_unit(wk, xk, kt[:, pr, c * SC : (c + 1) * SC], pr)
        wv = [wpool.tile([P, KO, HD], BF, tag="w", name="wv")]
        nc.sync.dma_start(wv[0], wvt.rearrange("(o p) m -> p o m", p=P))
        wq = [wpool.tile([P, KO, HD], BF, tag="w", name="wq")]
        nc.sync.dma_start(wq[0], wqt.rearrange("(o p) m -> p o m", p=P))
        xv = {0: load_x(xvt, 0, "xv0"), 1: load_x(xvt, 1, "xv1")}
        for u in range(8):
            if u == 4:
                xv[2] = load_x(xvt, 2, "xv2")
            v_unit(xv[u // 4], u)
        v_unit(xv[2], 8)
        v_unit(xv[2], 9)
        xq = {0: load_x(xqt, 0, "xq0", tag="xq8")}
        wo = wpool.tile([P, 4, D], BF, tag="w", name="wo")
        nc.sync.dma_start(wo, wot.rearrange("(pr p) n -> p pr n", p=P))
        qdone = set()
        v_issued = 10

        def q_unit(c, pr):
            qk_unit(wq, xq[c], qtt[c][pr], pr)
            qdone.add((c, pr))

        q_unit(0, 0)

        # ---------------- main pipelined attention loop ----------------
        exq = []  # FIFO of (ex, sk, meta); meta = [po, pi, ci]
        p3_backlog = []

        def norm(meta):
            po, pi, ci = meta
            a = att[ci][pi]
            pu = pupool.tile([65, 2 * JC], F32, tag="pu", name="pu")
            nc.vector.tensor_copy(pu, po)  # frees the po bank pair
            rb = rpool.tile([1, 2 * JC], F32, tag="rb", name="rb")
            nc.sync.dma_start(rb, pu[64:65, :])
            rc = rpool.tile([1, 2 * JC], F32, tag="rc", name="rc")
            nc.vector.reciprocal_approx_fast(out=rc, in_=rb)
            bc = bpool.tile([DK, 2 * JC], F32, tag="bc", name="bc")
            nc.gpsimd.partition_broadcast(bc, rc, channels=DK)
            nc.gpsimd.tensor_mul(a[0:DK, :], pu[0:DK, 0:JC], bc[:, 0:JC])
            tt = tpool.tile([DK, JC], BF, tag="tt", name="tt")
            nc.gpsimd.tensor_mul(tt, pu[0:DK, JC : 2 * JC], bc[:, JC : 2 * JC])
            nc.sync.dma_start(a[DK:P, :], tt)

        def pv_step():
            ex, sk, meta = exq.pop(0)
            if sk == 0:
                meta[0] = ps_out.tile([65, 2 * JC], F32, tag="po", name="po")
            po, pi, ci = meta
            ha, hb = 2 * pi, 2 * pi + 1
            last = sk == N_SK - 1
            nc.tensor.matmul(
                po[:, 0:JC], lhsT=va[:, sk, ha, :], rhs=ex[:, 0:JC],
                start=(sk == 0), stop=last,
            )
            nc.tensor.matmul(
                po[:, JC : 2 * JC], lhsT=va[:, sk, hb, :], rhs=ex[:, JC : 2 * JC],
                start=(sk == 0), stop=last,
            )
            if last:
                norm(meta)
                if pi == 3:
                    # consumable one full chunk after the (ci, p3) norm issues
                    ready = 4 * ci + 5
                    for st in range(4 * ci, 4 * ci + 4):
                        p3_backlog.append((st, 0, ready))
                        p3_backlog.append((st, 1, ready))

        for ci in range(N_C):
            for pi in range(4):
                X = 4 * ci + pi
                q = qtt[ci][pi]
                meta = [None, pi, ci]
                for sk in range(N_SK):
                    if (X, sk) in V_LATE:
                        u = V_LATE[(X, sk)]
                        if u % 4 == 0 and u // 4 not in xv:
                            xv[u // 4] = load_x(xvt, u // 4, f"xv{u // 4}")
                        v_unit(xv[u // 4], u)
                        v_issued = u + 1
                    if sk == 2:
                        nci, npi = (ci, pi + 1) if pi < 3 else (ci + 1, 0)
                        if nci < N_C and (nci, npi) not in qdone:
                            q_unit(nci, npi)
                    if sk == 7 and pi == 2 and ci + 1 < N_C and ci + 1 not in xq:
                        xq[ci + 1] = load_x(xqt, ci + 1, f"xq{ci + 1}", tag="xq8")
                    if sk in (5, 11) and p3_backlog and X >= p3_backlog[0][2]:
                        st_, half_, _ = p3_backlog.pop(0)
                        # Floor the scheduler's placement: the static cost
                        # model underestimates the normalize-chain latency and
                        # otherwise hoists these matmuls right behind the att
                        # write, stalling the in-order PE stream on the tt dma.
                        t_ms = (EST_START[X] + (EST_START[X + 1] - EST_START[X]) * sk / N_SK) / 1000.0
                        with tc.tile_wait_until(t_ms):
                            p3_unit(st_, half_)
                    pss = ps_score.tile([P, 2 * JC], F32, tag="pss", name="pss")
                    ks = slice(sk * P, (sk + 1) * P)
                    nc.tensor.matmul(
                        pss[:, 0:JC], lhsT=kt[0:DK, pi, ks], rhs=q[0:DK, :],
                        start=True, stop=True,
                    )
                    nc.tensor.matmul(
                        pss[:, JC : 2 * JC], lhsT=kt[DK:P, pi, ks], rhs=q[DK:P, :],
                        start=True, stop=True,
                    )
                    ex = epool.tile([P, 2 * JC], BF, tag="ex", name="ex")
                    nc.scalar.activation(ex, pss, EXP, scale=0.125)
                    exq.append((ex, sk, meta))
                    # pop PVs (up to 2 per step to drain deferral backlog);
                    # only issue a PV whose va sk-tile's unit is already issued
                    pops = 0
                    while (
                        len(exq) > LAG
                        and pops < 2
                        and (exq[0][1] < v_issued - 1 or v_issued == N_SK)
                    ):
                        pv_step()
                        pops += 1

        # tail: drain PV, then the last row's phase-3 units with the pr0-2
        # accumulations issued ahead of the pr3 (last-arriving) dependency.
        while exq:
            pv_step()
        tail = [(st, half) for st, half, _ in p3_backlog]
        p3_backlog.clear()
        pss_by_unit = {}
        for i, (st, half) in enumerate(tail[:2]):
            ps = ps_small.tile([P, 512], F32, tag="ps", name="ps_o")
            pss_by_unit[i] = ps
            p3_accum(ps, st, half, range(3))
        for i, (st, half) in enumerate(tail):
            ps = pss_by_unit[i]
            p3_accum(ps, st, half, [3])
            p3_finish(ps, st, half)
            if i + 2 < len(tail):
                st2, half2 = tail[i + 2]
                ps2 = ps_small.tile([P, 512], F32, tag="ps", name="ps_o")
                pss_by_unit[i + 2] = ps2
                p3_accum(ps2, st2, half2, range(3))

    nc.compile()
    return nc


_nc_cache = {}


def _get_nc(S_=2048):
    if S_ not in _nc_cache:
        _nc_cache[S_] = build()
    return _nc_cache[S_]


def _bf(a):
    return np.ascontiguousarray(a).astype(ml_dtypes.bfloat16)


def make_in_maps(Q, K, V, W_q, W_k, W_v, W_o):
    Q, K, V = (np.asarray(t, dtype=np.float32) for t in (Q, K, V))
    W_q, W_k, W_v, W_o = (np.asarray(t, dtype=np.float32) for t in (W_q, W_k, W_v, W_o))
    in_maps = []
    for c in range(8):
        b, hb = c // 2, c % 2
        sl = slice(hb * HD, (hb + 1) * HD)
        in_maps.append(
            {
                "xqt": _bf(Q[b].T),
                "xkt": _bf(K[b].T),
                "xvt": _bf(V[b].T),
                "wqt": _bf(W_q[sl, :].T),
                "wkt": _bf(W_k[sl, :].T),
                "wvt": _bf(W_v[sl, :].T),
                "wot": _bf(W_o[:, sl].T),
            }
        )
    return in_maps


def kernel(Q, K, V, W_q, W_k, W_v, W_o):
    nc = _get_nc(2048)
    in_maps = make_in_maps(Q, K, V, W_q, W_k, W_v, W_o)
    res = run_bass_kernel_spmd(nc, in_maps, core_ids=list(range(8)))
    outs = [res.results[c]["out"] for c in range(8)]
    full = np.stack([outs[2 * b] + outs[2 * b + 1] for b in range(4)], axis=0)
    return full.astype(np.float32)


# revision 29
# speedup vs baseline: 1.1706x; 1.1706x over previous
"""Trainium2 Bass kernel for nn_MultiHeadAttention (B=4, S=2048, D=1024, H=16).

Sharding: 8 cores = (batch b in 0..3) x (head-half hb in 0..1).
Each core computes, for its batch b and its 8 heads:
  qT = (Q[b] @ W_q[hb].T).T        [512, S]   (features on partitions)
  kT = (K[b] @ W_k[hb].T).T        [512, S]
  v  =  V[b] @ W_v[hb].T           [S, 512]   (+ ones column per head)
  per head pair: scoresT -> exp -> PV (ones-row gives softmax denominator
  in the same PSUM accumulation), normalize, then out-projection partial.
Host sums the two partial outputs per batch.

One software-pipelined schedule instead of serial phases:
  - prefix: K-proj (DMA-gated), 10/16 V-proj units, Q-proj(c0,p0)
  - main: 16 chunks (sq-chunk-major x head-pair). Score matmuls feed the
    ACT engine's exp (the true bottleneck, ~1.1us per [128,1024] tile);
    PV matmuls lag LAG sk-steps behind. The remaining V units, JIT Q-proj
    units and output-projection units interleave as PE filler.
  - tile_wait_until floors keep the BIR scheduler from hoisting the
    output-projection matmuls behind att writes (its static model
    underestimates the normalize-chain latency, stalling the in-order PE).
  - normalization: po drained to SBUF immediately (frees the single PSUM
    po bank pair), reciprocal_approx_fast (5x faster than reciprocal),
    broadcast + muls on the idle GPSIMD engine.
  - all matmul inputs bf16 (halves DMA; full PE rate), fp32 accumulate.
"""

import sys

sys.path.insert(0, "/opt/trn_rl_repo")

from contextlib import ExitStack

import ml_dtypes
import numpy as np

import concourse.bass as bass  # noqa: F401
import concourse.tile as tile
from concourse import bacc, mybir
from concourse.bass_utils import run_bass_kernel_spmd

F32 = mybir.dt.float32
BF = mybir.dt.bfloat16
EXP = mybir.ActivationFunctionType.Exp

D = 1024  # d_model
HD = 512  # head-dim slice per core (8 heads x 64)
DK = 64
NH = 8  # heads per core
P = 128
S = 2048
SC = 512  # proj streaming chunk of S
JC = 512  # sq chunk per head in attention
KO = D // P  # 8 contraction subtiles
N_SK = S // P  # 16 sk tiles
N_C = S // SC  # 4
LAG = 3  # PV lags exp by this many sk steps

# scheduler placement floors (approx sim-time, us)
PREFIX_US = 80.0
EST_START = [PREFIX_US + 17.8 * x for x in range(17)]

# V units deferred into the chunk loop: {(chunk_X, sk): unit index}
V_LATE = {}


def build():
    nc = bacc.Bacc(
        "TRN2",
        target_bir_lowering=False,
        debug=False,
        enable_asserts=False,
        num_devices=1,
    )

    xqt = nc.dram_tensor("xqt", [D, S], BF, kind="ExternalInput").ap()
    xkt = nc.dram_tensor("xkt", [D, S], BF, kind="ExternalInput").ap()
    xvt = nc.dram_tensor("xvt", [D, S], BF, kind="ExternalInput").ap()
    wqt = nc.dram_tensor("wqt", [D, HD], BF, kind="ExternalInput").ap()
    wkt = nc.dram_tensor("wkt", [D, HD], BF, kind="ExternalInput").ap()
    wvt = nc.dram_tensor("wvt", [D, HD], BF, kind="ExternalInput").ap()
    wot = nc.dram_tensor("wot", [HD, D], BF, kind="ExternalInput").ap()
    out = nc.dram_tensor("out", [S, D], F32, kind="ExternalOutput").ap()

    with tile.TileContext(nc) as tc, ExitStack() as ctx:
        pers = ctx.enter_context(tc.tile_pool(name="pers", bufs=1))
        wpool = ctx.enter_context(tc.tile_pool(name="wpool", bufs=3))
        xpool = ctx.enter_context(tc.tile_pool(name="xpool", bufs=2))
        epool = ctx.enter_context(tc.tile_pool(name="epool", bufs=6))
        pupool = ctx.enter_context(tc.tile_pool(name="pupool", bufs=3))
        rpool = ctx.enter_context(tc.tile_pool(name="rpool", bufs=3))
        bpool = ctx.enter_context(tc.tile_pool(name="bpool", bufs=3))
        tpool = ctx.enter_context(tc.tile_pool(name="tpool", bufs=3))
        opool = ctx.enter_context(tc.tile_pool(name="opool", bufs=4))
        ps_score = ctx.enter_context(tc.tile_pool(name="ps_score", bufs=2, space="PSUM"))
        ps_out = ctx.enter_context(tc.tile_pool(name="ps_out", bufs=1, space="PSUM"))
        ps_small = ctx.enter_context(tc.tile_pool(name="ps_small", bufs=2, space="PSUM"))

        # persistent intermediates. qt/at are split per (c-chunk, pair) tile so
        # concurrent JIT writers and readers get clean whole-tile dependencies.
        kt = pers.tile([P, 4, S], BF)  # kT: [p, pair, s], feature = pair*128+p
        va = pers.tile([P, N_SK, NH, DK + 1], BF)  # [s%128, s//128, h, dv|1]
        qtt = [
            [pers.tile([P, JC], BF, name=f"qt_{c}_{pr}") for pr in range(4)]
            for c in range(N_C)
        ]
        att = [
            [pers.tile([P, JC], BF, name=f"at_{c}_{pr}") for pr in range(4)]
            for c in range(N_C)
        ]

        nc.vector.memset(va[:, :, :, DK], 1.0)

        def load_x(src, c, nm, split=1, tag=None):
            # split>1 loads the chunk as `split` tiles along KO so the first
            # matmuls can start before the whole chunk lands.
            kw = KO // split
            xs = []
            for i in range(split):
                x = xpool.tile(
                    [P, kw, SC], BF, tag=tag or f"x{kw}", name=f"{nm}_{i}",
                    bufs=max(2, split),
                )
                nc.sync.dma_start(
                    x,
                    src[
                        i * kw * P : (i + 1) * kw * P, c * SC : (c + 1) * SC
                    ].rearrange("(o p) s -> p o s", p=P),
                )
                xs.append(x)
            return xs

        def qk_unit(ws, xs, dst, pr):
            ps = ps_small.tile([P, SC], F32, tag="ps", name="ps_qk")
            kw = KO // len(xs)
            kww = KO // len(ws)
            for k in range(KO):
                nc.tensor.matmul(
                    ps,
                    lhsT=ws[k // kww][:, k % kww, pr * P : (pr + 1) * P],
                    rhs=xs[k // kw][:, k % kw, :],
                    start=(k == 0),
                    stop=(k == KO - 1),
                )
            nc.vector.tensor_copy(dst, ps)

        def v_unit(xs, u):
            # unit u computes va[:, u] (sk tile u) from xv chunk u//4
            ps = ps_small.tile([P, HD], F32, tag="ps", name="ps_v")
            kw = KO // len(xs)
            st = u % 4
            for k in range(KO):
                nc.tensor.matmul(
                    ps,
                    lhsT=xs[k // kw][:, k % kw, st * P : (st + 1) * P],
                    rhs=wv[0][:, k, :],
                    start=(k == 0),
                    stop=(k == KO - 1),
                )
            nc.vector.tensor_copy(
                va[:, u, :, 0:DK],
                ps.rearrange("p (h d) -> p h d", d=DK),
            )

        def p3_accum(ps, st, half, prs):
            ci, so = st // 4, (st % 4) * P
            for pr in prs:
                nc.tensor.matmul(
                    ps,
                    lhsT=att[ci][pr][:, so : so + P],
                    rhs=wo[:, pr, half * 512 : (half + 1) * 512],
                    start=(pr == 0),
                    stop=(pr == 3),
                )

        def p3_finish(ps, st, half):
            ob = opool.tile([P, 512], F32, tag="ob", name="ob")
            nc.vector.tensor_copy(ob, ps)
            nc.sync.dma_start(out[st * P : (st + 1) * P, half * 512 : (half + 1) * 512], ob)

        def p3_unit(st, half):
            ps = ps_small.tile([P, 512], F32, tag="ps", name="ps_o")
            p3_accum(ps, st, half, range(4))
            p3_finish(ps, st, half)

        # ---------------- prefix ----------------
        # K-proj (DMA-gated), V units 0-9, Q(c0,p0). The first weight/x loads
        # are split so the PE starts sooner. bufs=2 x tags + the concurrent
        # DMA engine give one-chunk-deep prefetch.
        wk = []
        for i in range(2):
            w = wpool.tile([P, KO // 2, HD], BF, tag="wh", name=f"wk{i}", bufs=2)
            nc.sync.dma_start(
                w, wkt[i * 512 : (i + 1) * 512, :].rearrange("(o p) m -> p o m", p=P)
            )
            wk.append(w)
        for c in range(N_C):
            xk = load_x(xkt, c, f"xk{c}", split=4 if c == 0 else 1)
            for pr in range(4):
                qk_unit(wk, xk, kt[:, pr, c * SC : (c + 1) * SC], pr)
        wv = [wpool.tile([P, KO, HD], BF, tag="w", name="wv")]
        nc.sync.dma_start(wv[0], wvt.rearrange("(o p) m -> p o m", p=P))
        wq = [wpool.tile([P, KO, HD], BF, tag="w", name="wq")]
        nc.sync.dma_start(wq[0], wqt.rearrange("(o p) m -> p o m", p=P))
        xv = {}
        for u in range(16):
            if u % 4 == 0:
                xv[u // 4] = load_x(xvt, u // 4, f"xv{u // 4}")
            v_unit(xv[u // 4], u)
        xq = {0: load_x(xqt, 0, "xq0", tag="xq8")}
        wo = wpool.tile([P, 4, D], BF, tag="w", name="wo")
        nc.sync.dma_start(wo, wot.rearrange("(pr p) n -> p pr n", p=P))
        qdone = set()
        v_issued = 16

        def q_unit(c, pr):
            qk_unit(wq, xq[c], qtt[c][pr], pr)
            qdone.add((c, pr))

        q_unit(0, 0)

        # ---------------- main pipelined attention loop ----------------
        exq = []  # FIFO of (ex, sk, meta); meta = [po, pi, ci]
        p3_backlog = []

        def norm(meta):
            po, pi, ci = meta
            a = att[ci][pi]
            pu = pupool.tile([65, 2 * JC], F32, tag="pu", name="pu")
            nc.vector.tensor_copy(pu, po)  # frees the po bank pair
            rb = rpool.tile([1, 2 * JC], F32, tag="rb", name="rb")
            nc.sync.dma_start(rb, pu[64:65, :])
            rc = rpool.tile([1, 2 * JC], F32, tag="rc", name="rc")
            nc.vector.reciprocal_approx_fast(out=rc, in_=rb)
            bc = bpool.tile([DK, 2 * JC], F32, tag="bc", name="bc")
            nc.gpsimd.partition_broadcast(bc, rc, channels=DK)
            nc.gpsimd.tensor_mul(a[0:DK, :], pu[0:DK, 0:JC], bc[:, 0:JC])
            tt = tpool.tile([DK, JC], BF, tag="tt", name="tt")
            nc.gpsimd.tensor_mul(tt, pu[0:DK, JC : 2 * JC], bc[:, JC : 2 * JC])
            nc.sync.dma_start(a[DK:P, :], tt)

        def pv_step():
            ex, sk, meta = exq.pop(0)
            if sk == 0:
                meta[0] = ps_out.tile([65, 2 * JC], F32, tag="po", name="po")
            po, pi, ci = meta
            ha, hb = 2 * pi, 2 * pi + 1
            last = sk == N_SK - 1
            nc.tensor.matmul(
                po[:, 0:JC], lhsT=va[:, sk, ha, :], rhs=ex[:, 0:JC],
                start=(sk == 0), stop=last,
            )
            nc.tensor.matmul(
                po[:, JC : 2 * JC], lhsT=va[:, sk, hb, :], rhs=ex[:, JC : 2 * JC],
                start=(sk == 0), stop=last,
            )
            if last:
                norm(meta)
                if pi == 3:
                    # consumable one full chunk after the (ci, p3) norm issues
                    ready = 4 * ci + 5
                    for st in range(4 * ci, 4 * ci + 4):
                        p3_backlog.append((st, 0, ready))
                        p3_backlog.append((st, 1, ready))

        for ci in range(N_C):
            for pi in range(4):
                X = 4 * ci + pi
                q = qtt[ci][pi]
                meta = [None, pi, ci]
                for sk in range(N_SK):
                    if (X, sk) in V_LATE:
                        u = V_LATE[(X, sk)]
                        if u % 4 == 0 and u // 4 not in xv:
                            xv[u // 4] = load_x(xvt, u // 4, f"xv{u // 4}")
                        v_unit(xv[u // 4], u)
                        v_issued = u + 1
                    if sk == 2:
                        nci, npi = (ci, pi + 1) if pi < 3 else (ci + 1, 0)
                        if nci < N_C and (nci, npi) not in qdone:
                            q_unit(nci, npi)
                    if sk == 7 and pi == 2 and ci + 1 < N_C and ci + 1 not in xq:
                        xq[ci + 1] = load_x(xqt, ci + 1, f"xq{ci + 1}", tag="xq8")
                    if sk in (5, 11) and p3_backlog and X >= p3_backlog[0][2]:
                        st_, half_, _ = p3_backlog.pop(0)
                        # Floor the scheduler's placement: the static cost
                        # model underestimates the normalize-chain latency and
                        # otherwise hoists these matmuls right behind the att
                        # write, stalling the in-order PE stream on the tt dma.
                        t_ms = (EST_START[X] + (EST_START[X + 1] - EST_START[X]) * sk / N_SK) / 1000.0
                        with tc.tile_wait_until(t_ms):
                            p3_unit(st_, half_)
                    pss = ps_score.tile([P, 2 * JC], F32, tag="pss", name="pss")
                    ks = slice(sk * P, (sk + 1) * P)
                    nc.tensor.matmul(
                        pss[:, 0:JC], lhsT=kt[0:DK, pi, ks], rhs=q[0:DK, :],
                        start=True, stop=True,
                    )
                    nc.tensor.matmul(
                        pss[:, JC : 2 * JC], lhsT=kt[DK:P, pi, ks], rhs=q[DK:P, :],
                        start=True, stop=True,
                    )
                    ex = epool.tile([P, 2 * JC], BF, tag="ex", name="ex")
                    nc.scalar.activation(ex, pss, EXP, scale=0.125)
                    exq.append((ex, sk, meta))
                    # pop PVs (up to 2 per step to drain deferral backlog);
                    # only issue a PV whose va sk-tile's unit is already issued
                    pops = 0
                    while (
                        len(exq) > LAG
                        and pops < 2
                        and (exq[0][1] < v_issued - 1 or v_issued == N_SK)
                    ):
                        pv_step()
                        pops += 1

        # tail: drain PV, then the last row's phase-3 units with the pr0-2
        # accumulations issued ahead of the pr3 (last-arriving) dependency.
        while exq:
            pv_step()
        tail = [(st, half) for st, half, _ in p3_backlog]
        p3_backlog.clear()
        pss_by_unit = {}
        for i, (st, half) in enumerate(tail[:2]):
            ps = ps_small.tile([P, 512], F32, tag="ps", name="ps_o")
            pss_by_unit[i] = ps
            p3_accum(ps, st, half, range(3))
        for i, (st, half) in enumerate(tail):
            ps = pss_by_unit[i]
            p3_accum(ps, st, half, [3])
            p3_finish(ps, st, half)
            if i + 2 < len(tail):
                st2, half2 = tail[i + 2]
                ps2 = ps_small.tile([P, 512], F32, tag="ps", name="ps_o")
                pss_by_unit[i + 2] = ps2
                p3_accum(ps2, st2, half2, range(3))

    nc.compile()
    return nc


_nc_cache = {}


def _get_nc(S_=2048):
    if S_ not in _nc_cache:
        _nc_cache[S_] = build()
    return _nc_cache[S_]


def _bf(a):
    return np.ascontiguousarray(a).astype(ml_dtypes.bfloat16)


def make_in_maps(Q, K, V, W_q, W_k, W_v, W_o):
    Q, K, V = (np.asarray(t, dtype=np.float32) for t in (Q, K, V))
    W_q, W_k, W_v, W_o = (np.asarray(t, dtype=np.float32) for t in (W_q, W_k, W_v, W_o))
    in_maps = []
    for c in range(8):
        b, hb = c // 2, c % 2
        sl = slice(hb * HD, (hb + 1) * HD)
        in_maps.append(
            {
                "xqt": _bf(Q[b].T),
                "xkt": _bf(K[b].T),
                "xvt": _bf(V[b].T),
                "wqt": _bf(W_q[sl, :].T),
                "wkt": _bf(W_k[sl, :].T),
                "wvt": _bf(W_v[sl, :].T),
                "wot": _bf(W_o[:, sl].T),
            }
        )
    return in_maps


def kernel(Q, K, V, W_q, W_k, W_v, W_o):
    nc = _get_nc(2048)
    in_maps = make_in_maps(Q, K, V, W_q, W_k, W_v, W_o)
    res = run_bass_kernel_spmd(nc, in_maps, core_ids=list(range(8)))
    outs = [res.results[c]["out"] for c in range(8)]
    full = np.stack([outs[2 * b] + outs[2 * b + 1] for b in range(4)], axis=0)
    return full.astype(np.float32)
